# revision 14
# baseline (speedup 1.0000x reference)
"""MoE routing kernel for Trainium2 (8 NeuronCores, intermediate-sharded).

Problem: nn_MoDE_52140902973544 (moe_routing).
  x[4,2048,1024], router (8 experts, top-2, capacity 1024), 7 real experts
  with FFN H=1024 -> I=4096 -> H=1024 (relu), expert 7 = identity (noop).

Strategy:
  * Host: router forward + top-2 + capacity-limited dispatch (pure index
    math, order-based -> float-robust), gather dispatched tokens per
    expert transposed to [H, cap].
  * Device (SPMD over 8 cores): shard the FFN intermediate dim I=4096
    into 8 slices of 512.  Core c runs ALL 7 experts over its I-slice:
        h_e   = relu(Wi_e[:, c*512:(c+1)*512].T @ X_e.T)   # [512, cap]
        out_e = Wo_e[c*512:(c+1)*512, :].T @ h_e           # [H, cap] partial
    This balances PE work across all 8 cores (7/8 of the expert-parallel
    per-core load); relu is elementwise over I so slice-local relu is
    exact, and the partial outputs sum to the full FFN output.
  * Host: sum the 8 partial outputs, then combine via pure gathers
    (no scatter) + gate weights + noop path.

  Matmuls are issued in (k, n) order so both cap tiles share each
  128x128 stationary block, and _dedup_ldweights removes the then
  redundant InstLdweights (bass emits one per matmul unconditionally;
  the toolchain runs with --enable-ldw-opt=false, so each surviving
  Ldweights serializes ~128 PE cycles with the matmul stream, and the
  sparser Ldweights also keeps PE duty high enough that the HAM clock
  gate holds the array at full clock).
"""

import os
import sys

for _p in ("/opt/trn_rl_repo", "/opt/pypackages"):
    if _p not in sys.path:
        sys.path.append(_p)

import numpy as np

# ---- problem constants (hardcoded per contract) ----
B, S, H, I = 4, 2048, 1024, 4096
E = 8                 # experts incl. noop (last)
ER = E - 1            # real experts
TOP_K = 2
N_TOK = B * S         # 8192
CAP = 1024            # ceil(N_TOK / E * 1.0)
N_CORES = 8

P = 128               # partitions
KO = H // P           # 8   H chunks
IS = I // N_CORES     # 512 I-slice per core
KI = IS // P          # 4   I chunks per slice
NF = 512              # matmul free dim
NN = CAP // NF        # 2   cap tiles

MM_DTYPE = os.environ.get("MOE_MM_DTYPE", "bf16")

_CACHE = {}


def _dedup_ldweights(nc):
    """Drop redundant InstLdweights emitted for back-to-back matmuls that
    share the same stationary operand.

    bass emits an InstLdweights before EVERY InstMatmult, even when the
    weights AP is identical to the previous load and only matmuls (which
    don't disturb the loaded weights) ran in between.  With
    --enable-ldw-opt=false in the toolchain the redundant loads serialize
    with the matmul stream (~128 PE cycles each), so removing them is a
    direct PE-time win.  Conservative rules: only delete a Ldweights whose
    full signature (weights AP, tile size/position, perf mode, transpose)
    matches the previous PE weight load, with nothing but InstMatmult on
    the PE stream in between, and which carries no semaphore waits or
    updates of its own.
    """
    import concourse.mybir as mybir

    pe = mybir.EngineType.PE
    for blk in nc.main_func.blocks:
        instrs = blk.instructions
        last_key = None
        to_drop = []
        for idx, ins in enumerate(instrs):
            if getattr(ins, "engine", None) != pe:
                continue
            tname = type(ins).__name__
            if tname == "InstLdweights":
                si = ins.sync_info
                clean = si is None or (not si.on_wait and not si.on_update)
                key = (
                    str(ins.ins[0]),
                    str(getattr(ins, "tile_size", None)),
                    str(getattr(ins, "tile_position", None)),
                    str(getattr(ins, "perf_mode", None)),
                    str(getattr(ins, "is_transpose", None)),
                )
                if clean and last_key is not None and key == last_key:
                    to_drop.append(idx)
                else:
                    last_key = key
            elif tname != "InstMatmult":
                last_key = None
        for idx in reversed(to_drop):
            del instrs[idx]


def _build_nc(mm_dtype: str, repeat: int = 1):
    """Single-core Bass program (SPMD across 8 cores), I-sharded layout.

    Per-expert pipeline: DMA x_e / wiS_e / woS_e (double-buffered pools
    prefetch expert e+1 during e's compute), GEMM1 into 2 rotating PSUM
    banks with relu to bf16 h, GEMM2 (contraction KI=4) into 4 PSUM
    banks, copy to f32 out tile, DMA the partial out.  With repeat>1 the
    whole per-invocation body (all DMAs + compute) is repeated
    back-to-back, which a wall-clock slope over two repeat values turns
    into an honest per-invocation device time.
    """
    import concourse.bacc as bacc
    import concourse.mybir as mybir
    import concourse.tile as tile

    dt = mybir.dt
    assert mm_dtype == "bf16"
    DT = dt.bfloat16

    nc = bacc.Bacc("TRN2")
    xA = nc.declare_dram_parameter("xA", [ER * H, CAP], DT, isOutput=False)
    wiS = nc.declare_dram_parameter("wiS", [ER * H, IS], DT, isOutput=False)
    woS = nc.declare_dram_parameter("woS", [ER * IS, H], DT, isOutput=False)
    outP = nc.declare_dram_parameter("outP", [ER * H, CAP], dt.float32,
                                     isOutput=True)

    xA_r = xA.rearrange("(e ko p) n -> p e ko n", e=ER, p=P)    # [128,7,8,1024]
    wiS_r = wiS.rearrange("(e ko p) i -> p e ko i", e=ER, p=P)  # [128,7,8,512]
    woS_r = woS.rearrange("(e ki p) h -> p e ki h", e=ER, p=P)  # [128,7,4,1024]
    outP_r = outP.rearrange("(e ko p) n -> p e ko n", e=ER, p=P)

    # The Tile scheduler sometimes reorders independent PSUM accumulation
    # chains into chain-major order, splitting the (n0, n1) matmul pairs
    # that share a stationary block and forcing ~128 extra Ldweights
    # (576 vs the ideal 448).  PE executes sequentially, so pinning every
    # matmul after its predecessor with an ordering-only (nosync) dep
    # costs nothing and keeps all pairs adjacent for _dedup_ldweights.
    _prev_mm = [None]

    from concourse.instruction_name_ordered_set import InstructionNameOrderedSet

    def _mm(*args, **kw):
        h = nc.tensor.matmul(*args, **kw)
        if _prev_mm[0] is not None:
            h.ins.add_nosync_dependencies_from(
                InstructionNameOrderedSet([_prev_mm[0].ins.name]))
        _prev_mm[0] = h
        return h

    with tile.TileContext(nc) as tc:
        from contextlib import ExitStack

        with ExitStack() as ctx:
            xpool = ctx.enter_context(tc.tile_pool(name="x", bufs=2))
            wipool = ctx.enter_context(tc.tile_pool(name="wi", bufs=2))
            wopool = ctx.enter_context(tc.tile_pool(name="wo", bufs=2))
            hpool = ctx.enter_context(tc.tile_pool(name="h", bufs=2))
            opool = ctx.enter_context(tc.tile_pool(name="o", bufs=2))
            ps1pool = ctx.enter_context(
                tc.tile_pool(name="ps1", bufs=4, space="PSUM"))
            ps2pool = ctx.enter_context(
                tc.tile_pool(name="ps2", bufs=4, space="PSUM"))

            for _rep in range(repeat):
                for e in range(ER):
                    x_sb = xpool.tile([P, KO, CAP], DT, tag="x",
                                      name=f"x{_rep}_{e}")
                    nc.sync.dma_start(x_sb[:], xA_r[:, e])
                    wi_sb = wipool.tile([P, KO, IS], DT, tag="wi",
                                        name=f"wi{_rep}_{e}")
                    nc.sync.dma_start(wi_sb[:], wiS_r[:, e])
                    wo_sb = wopool.tile([P, KI, H], DT, tag="wo",
                                        name=f"wo{_rep}_{e}")
                    nc.sync.dma_start(wo_sb[:], woS_r[:, e])

                    h_sb = hpool.tile([P, KI, CAP], DT, tag="h",
                                      name=f"h{_rep}_{e}")
                    out_sb = opool.tile([P, KO, CAP], dt.float32, tag="o",
                                        name=f"o{_rep}_{e}")

                    # GEMM1: h = relu(WiS.T @ X.T)   [IS, CAP]
                    # (k, n) order: both cap tiles share each stationary so
                    # _dedup_ldweights halves the Ldweights count
                    for i in range(KI):
                        pts = [
                            ps1pool.tile([P, NF], dt.float32, tag="ps1",
                                         name=f"ps1_{_rep}_{e}_{i}_{n}")
                            for n in range(NN)
                        ]
                        for k in range(KO):
                            for n in range(NN):
                                _mm(
                                    pts[n][:],
                                    wi_sb[:, k, i * P:(i + 1) * P],
                                    x_sb[:, k, n * NF:(n + 1) * NF],
                                    start=(k == 0),
                                    stop=(k == KO - 1),
                                )
                        for n in range(NN):
                            nc.vector.tensor_scalar_max(
                                h_sb[:, i, n * NF:(n + 1) * NF], pts[n][:],
                                0.0)

                    # GEMM2: outP_e = WoS.T @ h      [H, CAP] (partial over I)
                    for mg in range(KO // 2):
                        pts = [
                            [
                                ps2pool.tile([P, NF], dt.float32, tag="ps2",
                                             name=f"ps2_{_rep}_{e}_{mg}_{m}_{n}")
                                for n in range(NN)
                            ]
                            for m in range(2)
                        ]
                        for k in range(KI):
                            for m in range(2):
                                for n in range(NN):
                                    _mm(
                                        pts[m][n][:],
                                        wo_sb[:, k,
                                              (mg * 2 + m) * P:(mg * 2 + m + 1) * P],
                                        h_sb[:, k, n * NF:(n + 1) * NF],
                                        start=(k == 0),
                                        stop=(k == KI - 1),
                                    )
                        for m in range(2):
                            for n in range(NN):
                                nc.vector.tensor_copy(
                                    out_sb[:, mg * 2 + m, n * NF:(n + 1) * NF],
                                    pts[m][n][:])

                    nc.sync.dma_start(outP_r[:, e], out_sb[:])
    _dedup_ldweights(nc)
    nc.compile()
    return nc


def _get_nc(mm_dtype: str):
    if mm_dtype not in _CACHE:
        _CACHE[mm_dtype] = _build_nc(mm_dtype)
    return _CACHE[mm_dtype]


def _routing(x_flat: np.ndarray, router_w: np.ndarray, router_b: np.ndarray):
    """Replicate the reference router bit-for-bit where possible (jax CPU),
    returning top-2 values/indices [N_TOK, 2] (fp32/int)."""
    try:
        import jax
        import jax.numpy as jnp

        cpu = jax.devices("cpu")[0]
        with jax.default_device(cpu):
            xj = jnp.asarray(x_flat.reshape(B, S, H))
            logits = jnp.einsum("bsh,eh->bse", xj, jnp.asarray(router_w)) \
                + jnp.asarray(router_b)
            wflat = jax.nn.softmax(logits, axis=-1).reshape(N_TOK, E)
            topv, topi = jax.lax.top_k(wflat, TOP_K)
            return np.asarray(topv), np.asarray(topi)
    except Exception:
        # numpy fallback (float64 logits for a stable ordering)
        logits = x_flat.astype(np.float64) @ router_w.astype(np.float64).T \
            + router_b.astype(np.float64)
        m = logits.max(axis=1, keepdims=True)
        ex = np.exp(logits - m)
        wflat = (ex / ex.sum(axis=1, keepdims=True)).astype(np.float32)
        topi = np.argsort(-wflat, axis=1, kind="stable")[:, :TOP_K]
        topv = np.take_along_axis(wflat, topi, axis=1)
        return topv, topi


def _dispatch(x_flat, topv, topi):
    """Capacity-limited dispatch: per-expert token lists (first CAP in
    token order), gathered and transposed to [ER, H, CAP]."""
    mask = np.zeros((N_TOK, E), dtype=bool)
    rows = np.arange(N_TOK)
    mask[rows[:, None], topi] = True
    expert_mask = mask[:, :ER]                       # [N, 7]
    pos = np.cumsum(expert_mask, axis=0, dtype=np.int32) - 1

    disp_T = np.zeros((ER, H, CAP), dtype=np.float32)
    for e in range(ER):
        idx_e = np.nonzero(expert_mask[:, e])[0][:CAP]
        disp_T[e, :, :len(idx_e)] = x_flat[idx_e].T
    return disp_T, pos


def _in_maps(disp_T, experts_inter, experts_out):
    import ml_dtypes

    bf = lambda a: np.ascontiguousarray(a.astype(ml_dtypes.bfloat16))
    xA = bf(disp_T.reshape(ER * H, CAP))
    maps = []
    for c in range(N_CORES):
        maps.append({
            "xA": xA,
            "wiS": bf(experts_inter[:, :, c * IS:(c + 1) * IS].reshape(
                ER * H, IS)),
            "woS": bf(experts_out[:, c * IS:(c + 1) * IS, :].reshape(
                ER * IS, H)),
        })
    return maps


def kernel(x, router_w, router_b, experts_inter, experts_out):
    from concourse.bass_utils import run_bass_kernel_spmd

    x = np.ascontiguousarray(np.asarray(x, dtype=np.float32))
    router_w = np.asarray(router_w, dtype=np.float32)
    router_b = np.asarray(router_b, dtype=np.float32)
    experts_inter = np.asarray(experts_inter, dtype=np.float32)
    experts_out = np.asarray(experts_out, dtype=np.float32)

    x_flat = x.reshape(N_TOK, H)
    topv, topi = _routing(x_flat, router_w, router_b)
    disp_T, pos = _dispatch(x_flat, topv, topi)
    in_maps = _in_maps(disp_T, experts_inter, experts_out)

    nc = _get_nc(MM_DTYPE)
    res = run_bass_kernel_spmd(nc, in_maps, list(range(N_CORES)))
    global LAST_RESULT
    LAST_RESULT = res

    # sum the 8 partial outputs (I-shards) -> full expert outputs [7,H,cap]
    out_T = np.zeros((ER, H, CAP), dtype=np.float32)
    for c in range(N_CORES):
        out_T += res.results[c]["outP"].reshape(ER, H, CAP)

    # ---- host combine: pure gathers ----
    out_flat = np.ascontiguousarray(out_T.transpose(0, 2, 1)).reshape(
        ER * CAP, H)
    out_ext = np.vstack([out_flat, np.zeros((1, H), dtype=np.float32)])

    rows = np.arange(N_TOK)
    combined = np.zeros_like(x_flat)
    noop_w = np.zeros(N_TOK, dtype=np.float32)
    for k in range(TOP_K):
        e_k = topi[:, k]
        v_k = topv[:, k]
        is_noop = e_k == ER
        noop_w += np.where(is_noop, v_k, 0.0).astype(np.float32)
        p_k = pos[rows, np.minimum(e_k, ER - 1)]
        ok = (~is_noop) & (p_k < CAP)
        slot = np.where(ok, np.minimum(e_k, ER - 1) * CAP + p_k, ER * CAP)
        combined += out_ext[slot] * np.where(ok, v_k, 0.0)[:, None]
    combined += x_flat * noop_w[:, None]

    return combined.reshape(B, S, H)
